# revision 48
# baseline (speedup 1.0000x reference)
"""Causal multi-head attention (B=4, S=2048, D=1024, H=16, hd=64) on 8 TRN2
NeuronCores.

Sharding: core c = (batch b = c//2, head-group g = c%2). Each core computes
QKV projections for its 8 heads (Megatron column-split), causal attention,
and a partial out-projection (row-split); the host sums the two head-group
partials per batch and adds the bias.

On-device layout (bf16 compute, fp32 PSUM accumulation):
  xT  [p, q-block, din-subtile, 512]  x[b]^T pre-tiled on host so each
        input DMA moves 8KB-contiguous runs per partition (descriptor-
        efficient); same for wq/wk (head-pair-blocked), wv, wo
  q/k projections in fp8-e4m3 DoubleRow (weights pre-scaled x64 on host,
        the exp scale absorbs the 1/4096; value path stays bf16 - fp8
        anywhere in v/attn costs ~1% extra rel err, over budget)
  qT/kT as [d_g, S] transposed tiles: head-pair t -> partitions
        [0:64] head 2t, [64:128] head 2t+1
  v   [k-tile 128, 8 heads, 65]: col 64 is ones (sumexp lands in the ctx^T
        psum row 64 for free during the attn*V matmul)
  scores^T psum tiles [k 128, 2 heads, q 512]: head pair packed via PE row
        tiling (K=64 each, concurrent).  Scores for TWO k-tiles are emitted
        back-to-back: full-array<->row-group LDWEIGHTS transitions stall
        ~100ns each (the PE can only pull an LDW ahead of in-flight matmuls
        into a non-conflicting row group), so batching the row-tiled pairs
        halves the number of transitions.
  attn = exp(scores/8) per k-tile on ScalarE; causal via skipping k-tiles
        above the diagonal, restricting the q-range on diagonal tiles, and
        one masked multiply per diagonal tile (the 128-wide diagonal block
        is the same upper triangle for every dd, both heads in one op)
  ctx^T accumulated in PSUM over k-tiles; normalize via DRAM-roundtrip
        reciprocal + gpsimd partition broadcast.  The last chunk instead
        transposes the sumexp row straight out of PSUM (DVE 32x32 block
        transpose), reciprocates in place, transposes back, and multiplies
        straight from PSUM - no SBUF copies on the critical tail.

Schedule: all non-attention matmuls drain as filler INSIDE the attention
stream (interpolated between per-chunk prerequisite markers); attn*V is
emitted one k-tile-PAIR late so its exp is always ready; ~10 short dummy
matmuls keep HAM at full clock through the initial DMA wait without
head-blocking the prefill; input DMAs are spread over the sync, scalar and
gpsimd rings so the scalar queue is clear before the exp stream starts;
row-3 out-projection is split so only one matmul + add + DMA per o-tile
remains after the final normalize, pipelined over psum/sbuf double-buffers
and both DMA rings.
"""

import numpy as np
import ml_dtypes

import concourse.bass as bass
import concourse.tile as tile
from concourse import bacc, mybir
from concourse.bass_utils import run_bass_kernel_spmd

P = 128          # partitions
S = 2048         # sequence length (one batch per core)
DIN = 1024       # model dim
DG = 512         # head-group width per core (8 heads x 64)
HD = 64          # head dim
NH = 8           # heads per core
QC = 512         # q-chunk (matmul free dim)
NQC = S // QC    # 4 q-chunks
NKT = S // P     # 16 k-tiles
KDT = DIN // P   # 8 din k-tiles
NHP = 4          # head pairs per core
F32 = mybir.dt.float32
BF16 = mybir.dt.bfloat16
FP8 = mybir.dt.float8e4
EXP = mybir.ActivationFunctionType.Exp
DR = mybir.MatmulPerfMode.DoubleRow

WSCALE = 64.0    # fp8 q/k weight pre-scale
N_WARM = 10      # dummy warm-up matmuls during the initial DMA wait; must
WARM_N = 512     # sustain >3.4us of PE activity to trip HAM to full clock
N_TAILWARM = 20  # dummy matmuls covering the last-normalize PE gap

_CACHE = {}


def _emit(tc, d):
    nc = tc.nc
    with (
        nc.allow_low_precision(reason="bf16 attention pipeline"),
        tc.tile_pool(name="persist", bufs=1) as pp,
        tc.tile_pool(name="work", bufs=4) as wp,
        tc.tile_pool(name="psc", bufs=2, space="PSUM") as psc,
        tc.tile_pool(name="ppj", bufs=2, space="PSUM") as ppj,
        tc.tile_pool(name="pcx", bufs=1, space="PSUM") as pcx,
    ):
        # ---- persistent SBUF tiles (layouts match the pre-tiled DRAM) ----
        xT = pp.tile([P, NQC, KDT, QC], BF16, tag="xT", name="xT")
        x8 = pp.tile([P, NQC, KDT, QC], FP8, tag="x8", name="x8")
        wq = pp.tile([P, NHP, KDT, P], FP8, tag="wq", name="wq")
        wk = pp.tile([P, NHP, KDT, P], FP8, tag="wk", name="wk")
        wv = pp.tile([P, KDT, DG], BF16, tag="wv", name="wv")
        wo = pp.tile([P, 4, DIN], BF16, tag="wo", name="wo")
        qT = [pp.tile([P, S], BF16, tag=f"qT{t}", name=f"qT{t}") for t in range(NHP)]
        kT = [pp.tile([P, S], BF16, tag=f"kT{t}", name=f"kT{t}") for t in range(NHP)]
        vv = [pp.tile([P, NH, HD + 1], BF16, tag=f"v{m}", name=f"v{m}") for m in range(NKT)]
        cx = [pp.tile([P, S], BF16, tag=f"cx{t}", name=f"cx{t}") for t in range(NHP)]
        ob3 = pp.tile([P, 8, QC], BF16, tag="ob3", name="ob3")
        msk = pp.tile([P, 2, P], BF16, tag="msk", name="msk")
        idt = pp.tile([P, P], BF16, tag="idt", name="idt")
        wrm = pp.tile([P, WARM_N], BF16, tag="wrm", name="wrm")

        # ---- PE warm-up: garbage matmuls (the psum is never read) keep the
        # PE busy from the earliest possible moment so HAM reaches full
        # clock before the first chains, without committing the PE FIFO
        # much past the x8 arrival ----
        nc.vector.memset(wrm[:], 0.0)
        for g in range(0, N_WARM, 5):
            ps = ppj.tile([P, WARM_N], F32, tag="pj", name="ps")
            n = min(5, N_WARM - g)
            for i in range(n):
                nc.tensor.matmul(
                    ps[:], wrm[:, 0:P], wrm[:],
                    start=(i == 0), stop=(i == n - 1),
                )

        # ---- input DMAs: big contiguous-run transfers, ordered by need,
        # critical prefill set first, spread across three HW-DGE rings so
        # the scalar queue is idle before the exp stream starts ----
        # quarter-granular x8 chunk: each quarter is exactly one DR matmul's
        # operand, so the prefill chains start as soon as the FIRST quarter
        # lands instead of waiting for a 256KB half
        for qq in range(0, KDT, 2):
            nc.sync.dma_start(x8[:, 0, qq:qq + 2, :], d["xq"][:, 0, qq:qq + 2, :])
        nc.scalar.dma_start(wq[:, 0, :, :], d["wqT"][:, 0, :, :])
        nc.scalar.dma_start(wk[:, 0, :, :], d["wkT"][:, 0, :, :])
        nc.scalar.dma_start(msk[:], d["masks"][:])
        nc.sync.dma_start(wv[:], d["wvT"][:])
        nc.sync.dma_start(xT[:, 0, :, :], d["xT"][:, 0, :, :])
        nc.scalar.dma_start(wq[:, 1:NHP, :, :], d["wqT"][:, 1:NHP, :, :])
        nc.scalar.dma_start(wk[:, 1:NHP, :, :], d["wkT"][:, 1:NHP, :, :])
        nc.scalar.dma_start(idt[:], d["ident"][:])
        # deferred inputs ride the scalar ring BEHIND the critical weights:
        # the serial ring is a priority queue, while a separate (empty) ring
        # would fire immediately and steal HBM bandwidth from the critical
        # x8/xT stream on the sync ring
        for s in range(1, NQC):
            nc.sync.dma_start(xT[:, s, :, :], d["xT"][:, s, :, :])
            nc.scalar.dma_start(x8[:, s, :, :], d["xq"][:, s, :, :])
        nc.scalar.dma_start(wo[:], d["woT"][:])

        # ---- filler units ----
        def u_v(m):
            def f():
                ps = ppj.tile([P, QC], F32, tag="pj", name="ps")
                for k in range(KDT):
                    nc.tensor.matmul(
                        ps[:],
                        xT[:, m // 4, k, (m % 4) * P:(m % 4 + 1) * P],
                        wv[:, k, :],
                        start=(k == 0),
                        stop=(k == KDT - 1),
                    )
                nc.vector.tensor_copy(
                    vv[m][:, :, 0:HD], ps[:].rearrange("p (h e) -> p h e", h=NH)
                )
                nc.vector.memset(vv[m][:, :, HD:HD + 1], 1.0)
            return f

        def u_chain(t, w, s):
            def f():
                wt, dst = ((wq, qT), (wk, kT))[w]
                ps = ppj.tile([P, QC], F32, tag="pj", name="ps")
                for k in range(0, KDT, 2):
                    nc.tensor.matmul(
                        ps[:],
                        wt[:, t, k:k + 2, :],
                        x8[:, s, k:k + 2, :],
                        start=(k == 0),
                        stop=(k == KDT - 2),
                        perf_mode=DR,
                    )
                nc.vector.tensor_copy(dst[t][:, s * QC:(s + 1) * QC], ps[:])
            return f

        def u_out(s, o):
            def f():
                ps = ppj.tile([P, QC], F32, tag="pj", name="ps")
                for k in range(4):
                    nc.tensor.matmul(
                        ps[:],
                        wo[:, k, o * P:(o + 1) * P],
                        cx[k][:, s * QC:(s + 1) * QC],
                        start=(k == 0), stop=(k == 3),
                    )
                ob = wp.tile([P, QC], BF16, tag="ob", name="ob", bufs=2)
                nc.vector.tensor_copy(ob[:], ps[:])
                nc.sync.dma_start(
                    d["outT"][o * P:(o + 1) * P, s * QC:(s + 1) * QC], ob[:]
                )
            return f

        def u_out3_partial(o):
            # row-3 out-proj, head-pair groups 0..2 only -> SBUF partial
            def f():
                ps = ppj.tile([P, QC], F32, tag="pj", name="ps")
                for k in range(3):
                    nc.tensor.matmul(
                        ps[:],
                        wo[:, k, o * P:(o + 1) * P],
                        cx[k][:, 3 * QC:S],
                        start=(k == 0), stop=(k == 2),
                    )
                nc.vector.tensor_copy(ob3[:, o, :], ps[:])
            return f

        def u_out3_final(t):
            # o-tile pair 2t/2t+1: two row-3 matmuls + ONE identity matmul
            # folding in the row-0..2 partials (PE is idle at the tail, DVE
            # is not); psum->SBUF copies alternate DVE/ScalarE so the two
            # copy chains run in parallel
            ps = psc.tile([P, 2, QC], F32, tag="sc", name="fin")
            for j in range(2):
                o = 2 * t + j
                nc.tensor.matmul(
                    ps[:, j, :], wo[:, 3, o * P:(o + 1) * P], cx[3][:, 3 * QC:S],
                    start=True, stop=False,
                )
                nc.tensor.matmul(
                    ps[:, j, :], idt[:], ob3[:, o, :], start=False, stop=True,
                )
            # half-granular copies so each o-tile's DMA fires as soon as its
            # half of the psum is copied out (copy chains alternate engines)
            ob = wp.tile([P, 2, QC], BF16, tag="obf", name="obf", bufs=2)
            eng = nc.sync if t % 2 == 0 else nc.gpsimd
            for j in range(2):
                o = 2 * t + j
                if t % 2 == 0:
                    nc.vector.tensor_copy(ob[:, j, :], ps[:, j, :])
                else:
                    nc.scalar.activation(
                        ob[:, j, :], ps[:, j, :],
                        mybir.ActivationFunctionType.Copy,
                    )
                eng.dma_start(
                    d["outT"][o * P:(o + 1) * P, 3 * QC:S], ob[:, j, :]
                )

        # consume-ordered filler queue + hard prerequisites per chunk
        queue = [u_v(0), u_v(1), u_v(2), u_v(3)]
        pre = {}
        for s in range(NQC):
            for hp in range(NHP):
                if (hp, s) == (0, 0):
                    pre[(hp, s)] = 0
                    continue
                if hp == 0 and s >= 1:
                    queue += [u_v(m) for m in range(4 * s, 4 * s + 4)]
                queue += [u_chain(hp, 0, s), u_chain(hp, 1, s)]
                pre[(hp, s)] = len(queue)
            if s == 1 or s == 2:
                queue += [u_out(s - 1, o) for o in range(8)]
        queue += [u_out(2, o) for o in range(8)]
        queue += [u_out3_partial(o) for o in range(8)]
        n_units = len(queue)

        order = [(hp, s) for s in range(NQC) for hp in range(NHP)]
        nxt = {order[i]: order[i + 1] for i in range(len(order) - 1)}

        state = {"drained": 0}

        def drain_to(idx):
            while state["drained"] < idx:
                queue[state["drained"]]()
                state["drained"] += 1

        def normalize(hp, s):
            last = (hp, s) == (NHP - 1, NQC - 1)
            cps = state["cps"]
            if last:
                # low-latency tail: DVE 32x32 block-transpose lifts the
                # [1,1024] sumexp row (row 64 = col 0 of the 32-aligned psum
                # window [64:96]; rows 65:96 are memset filler) onto 32
                # partitions straight out of PSUM, reciprocal in place
                # (col 0 -> col 1), transpose back, then multiply straight
                # from PSUM - no SBUF staging on the critical tail
                t1 = wp.tile([32, 2, QC], F32, tag="t1", name="t1", bufs=1)
                t2 = wp.tile([32, 2, QC], F32, tag="t2", name="t2", bufs=1)
                nc.vector.memset(t2[:], 1.0)  # early, off the critical path
                nc.vector.transpose(t1[:], cps[64:96, :, :])
                tv1 = t1[:].rearrange("p h (b j) -> p h b j", j=32)
                tv2 = t2[:].rearrange("p h (b j) -> p h b j", j=32)
                nc.vector.reciprocal(tv2[:, :, :, 0:1], tv1[:, :, :, 0:1])
                rc = wp.tile([32, 2, QC], F32, tag="rc", name="rc", bufs=1)
                nc.vector.transpose(rc[:], t2[:])
                bs = wp.tile([HD, 2, QC], F32, tag="bs", name="bs", bufs=2)
                nc.gpsimd.partition_broadcast(bs[:], rc[0:1, :, :])
                state["bs"] = bs
                cxs = wp.tile([HD, QC], BF16, tag="cxs", name="cxs", bufs=2)
                nc.vector.tensor_mul(cxs[:], cps[0:HD, 1, :], bs[:, 1, :])
                nc.sync.dma_start(cx[hp][HD:P, s * QC:(s + 1) * QC], cxs[:])
                nc.vector.tensor_mul(
                    cx[hp][0:HD, s * QC:(s + 1) * QC], cps[0:HD, 0, :], bs[:, 0, :]
                )
                return
            cb = wp.tile([HD + 1, 2, QC], F32, tag="cb", name="cb", bufs=2)
            nc.vector.tensor_copy(cb[:], cps[0:HD + 1, :, :])
            zt = wp.tile([P, 8], F32, tag="zt", name="zt", bufs=2)
            nc.sync.dma_start(zt[:], cb[HD:HD + 1, :, :])
            rt = wp.tile([P, 8], F32, tag="rt", name="rt", bufs=2)
            nc.vector.reciprocal(rt[:], zt[:])
            rr = wp.tile([1, 2, QC], F32, tag="rr", name="rr", bufs=2)
            nc.sync.dma_start(rr[:], rt[:])
            bs = wp.tile([HD, 2, QC], F32, tag="bs", name="bs", bufs=2)
            nc.gpsimd.partition_broadcast(bs[:], rr[:])
            # head B first: its partition-shift DMA overlaps head A's mul
            cxs = wp.tile([HD, QC], BF16, tag="cxs", name="cxs", bufs=2)
            nc.vector.tensor_mul(cxs[:], cb[0:HD, 1, :], bs[:, 1, :])
            nc.sync.dma_start(cx[hp][HD:P, s * QC:(s + 1) * QC], cxs[:])
            nc.vector.tensor_mul(
                cx[hp][0:HD, s * QC:(s + 1) * QC], cb[0:HD, 0, :], bs[:, 0, :]
            )

        def attn_chunk(hp, s):
            t0 = pre[(hp, s)]
            t1 = pre[nxt[(hp, s)]] if (hp, s) in nxt else n_units
            nkt = 4 * (s + 1)  # causal: k-tiles 0..nkt-1
            cps = pcx.tile([96, 2, QC], F32, tag="cx", name="cps")
            state["cps"] = cps
            if (hp, s) == (NHP - 1, NQC - 1):
                # valid filler above the sumexp row for the tail transpose
                # (32-aligned window; row 64 is re-written by the start=True
                # attn*V accumulation right after)
                nc.vector.memset(cps[HD:96, :, :], 1.0)

            def attn_v_pair(pair):
                for k, s0, a in pair:
                    nc.tensor.matmul(
                        cps[0:HD + 1, 0, s0:], vv[k][:, 2 * hp, :], a[:, 0, s0:],
                        start=(k == 0), stop=(k == nkt - 1),
                    )
                    nc.tensor.matmul(
                        cps[0:HD + 1, 1, s0:], vv[k][:, 2 * hp + 1, :], a[:, 1, s0:],
                        start=(k == 0), stop=(k == nkt - 1),
                    )

            pend = None  # attn*V emitted one k-tile-PAIR late: its exp and
            # mask are always done by the time it reaches the head of the
            # FIFO tensor queue, so it never head-blocks the scores stream
            for pk in range(nkt // 2):
                cur = []
                for k in (2 * pk, 2 * pk + 1):
                    dd = k - 4 * s
                    s0 = max(dd, 0) * P  # causal q-range restriction
                    sps = psc.tile([P, 2, QC], F32, tag="sc", name="sps")
                    nc.tensor.matmul(
                        sps[:, 0, s0:],
                        kT[hp][0:HD, k * P:(k + 1) * P],
                        qT[hp][0:HD, s * QC + s0:(s + 1) * QC],
                        start=True, stop=True,
                    )
                    nc.tensor.matmul(
                        sps[:, 1, s0:],
                        kT[hp][HD:P, k * P:(k + 1) * P],
                        qT[hp][HD:P, s * QC + s0:(s + 1) * QC],
                        start=True, stop=True,
                    )
                    a = wp.tile([P, 2, QC], BF16, tag="a", name="a", bufs=5)
                    nc.scalar.activation(
                        a[:, :, s0:], sps[:, :, s0:], EXP, scale=d["escale"]
                    )
                    if dd >= 0:
                        # only columns [s0, s0+128) straddle the diagonal;
                        # the block mask is the same triangle for every dd
                        nc.vector.tensor_mul(
                            a[:, :, s0:s0 + P], a[:, :, s0:s0 + P], msk[:]
                        )
                    cur.append((k, s0, a))
                if pend is not None:
                    attn_v_pair(pend)
                pend = cur
                drain_to(min(t1, t0 + ((t1 - t0) * (2 * pk + 4)) // nkt,
                             state["drained"] + 5))
            attn_v_pair(pend)
            normalize(hp, s)

        # ---- prefill: just enough to start chunk (0,0) ----
        # x8-quarter-gated garbage DR matmuls first: each becomes ready as
        # its DMA quarter lands, keeping the PE activity window busy (HAM at
        # 8/8) across the warmup-to-chain transition
        for qq in range(0, KDT, 2):
            pw = ppj.tile([P, QC], F32, tag="pj", name="dumq")
            nc.tensor.matmul(
                pw[:], x8[:, 0, qq:qq + 2, 0:P], x8[:, 0, qq:qq + 2, :],
                start=True, stop=True, perf_mode=DR,
            )
        u_chain(0, 0, 0)()
        u_chain(0, 1, 0)()

        # ---- main stream ----
        for s in range(NQC):
            for hp in range(NHP):
                drain_to(pre[(hp, s)])
                attn_chunk(hp, s)
        drain_to(n_units)
        # dummy matmuls bridge the PE gap while the final normalize runs so
        # HAM stays at full clock for the final out-projection row
        # the first dummy groups use the scores psum ring (free right after
        # the last exp) so they are not serialized behind the last partials'
        # psum copies on the ppj ring
        for g in range(0, N_TAILWARM, 5):
            if g < 10:
                ps = psc.tile([P, 2, QC], F32, tag="sc", name="dum")
            else:
                ps = ppj.tile([P, QC], F32, tag="pj", name="dum")
            n = min(5, N_TAILWARM - g)
            for i in range(n):
                nc.tensor.matmul(
                    ps[:, 0, :] if g < 10 else ps[:], wrm[:, 0:P], wrm[:],
                    start=(i == 0), stop=(i == n - 1),
                )
        # two f32 garbage matmuls gated on the final broadcast: they become
        # ready exactly when it lands, bridging the remaining PE idle window
        # up to the final out-projection (keeps HAM at 8/8); emitted AFTER
        # the plain dummies so they never head-block the FIFO
        bs = state["bs"]
        for _ in range(3):
            pw = ppj.tile([P, QC], F32, tag="pj", name="dumb")
            nc.tensor.matmul(
                pw[:], bs[:, 0, 0:P], bs[:, 0, :], start=True, stop=True,
            )
        for t in range(4):
            u_out3_final(t)


def _build():
    if "nc" in _CACHE:
        return _CACHE["nc"]
    nc = bacc.Bacc("TRN2", target_bir_lowering=False, debug=False, num_devices=8)
    d = {
        "xT": nc.dram_tensor("xT", [P, NQC, KDT, QC], BF16, kind="ExternalInput").ap(),
        "wvT": nc.dram_tensor("wvT", [P, KDT, DG], BF16, kind="ExternalInput").ap(),
        "woT": nc.dram_tensor("woT", [P, 4, DIN], BF16, kind="ExternalInput").ap(),
        "masks": nc.dram_tensor("masks", [P, 2, P], BF16, kind="ExternalInput").ap(),
        "ident": nc.dram_tensor("ident", [P, P], BF16, kind="ExternalInput").ap(),
        "outT": nc.dram_tensor("outT", [DIN, S], BF16, kind="ExternalOutput").ap(),
        "wqT": nc.dram_tensor("wqT", [P, NHP, KDT, P], FP8, kind="ExternalInput").ap(),
        "wkT": nc.dram_tensor("wkT", [P, NHP, KDT, P], FP8, kind="ExternalInput").ap(),
        "xq": nc.dram_tensor("xq", [P, NQC, KDT, QC], FP8, kind="ExternalInput").ap(),
        "escale": 0.125 / (WSCALE * WSCALE),
    }
    with tile.TileContext(nc) as tc:
        _emit(tc, d)
    nc.compile()
    _CACHE["nc"] = nc
    return nc


def _masks_np():
    r = np.arange(P)[:, None]
    j = np.arange(P)[None, :]
    m = (j >= r).astype(ml_dtypes.bfloat16)  # [128, 128] upper triangle
    return np.ascontiguousarray(np.broadcast_to(m[:, None, :], (P, 2, P)))


def _tile_k(a, kdt=KDT):
    """[kdt*P, C] -> [P, kdt, C] (din-subtile blocking)."""
    c = a.shape[1]
    return np.ascontiguousarray(a.reshape(kdt, P, c).transpose(1, 0, 2))


def _f8(a):
    return np.clip(a, -240, 240).astype(ml_dtypes.float8_e4m3)


def kernel(x, Wq, Wk, Wv, Wo, bo, _run_kwargs=None, _return_res=False):
    x = np.asarray(x)
    Wq, Wk, Wv, Wo, bo = (np.asarray(a) for a in (Wq, Wk, Wv, Wo, bo))
    B = x.shape[0]
    nc = _build()

    def b16(a):
        return np.ascontiguousarray(a).astype(ml_dtypes.bfloat16)

    masks = _masks_np()
    in_maps = []
    for c in range(8):
        b, g = divmod(c, 2)
        xt = b16(x[b].T)  # [1024, 2048]
        xt4 = xt.reshape(KDT, P, NQC, QC).transpose(1, 2, 0, 3)  # [p,s,k,c]
        wqt = Wq[g * DG:(g + 1) * DG, :].T  # [1024, 512] f32
        wkt = Wk[g * DG:(g + 1) * DG, :].T
        im = {
            "xT": np.ascontiguousarray(xt4),
            "wvT": _tile_k(b16(Wv[g * DG:(g + 1) * DG, :].T)),
            "woT": _tile_k(b16(Wo[:, g * DG:(g + 1) * DG].T), kdt=4),
            "masks": masks,
            "ident": np.eye(P, dtype=ml_dtypes.bfloat16),
            "xq": np.ascontiguousarray(
                _f8(x[b].T).reshape(KDT, P, NQC, QC).transpose(1, 2, 0, 3)),
            "wqT": np.ascontiguousarray(
                _f8(WSCALE * wqt).reshape(KDT, P, NHP, P).transpose(1, 2, 0, 3)),
            "wkT": np.ascontiguousarray(
                _f8(WSCALE * wkt).reshape(KDT, P, NHP, P).transpose(1, 2, 0, 3)),
        }
        in_maps.append(im)

    res = run_bass_kernel_spmd(nc, in_maps, list(range(8)), **(_run_kwargs or {}))
    out = np.empty((B, S, DIN), np.float32)
    for b in range(B):
        p = (res.results[2 * b]["outT"].astype(np.float32)
             + res.results[2 * b + 1]["outT"].astype(np.float32))
        out[b] = p.T + bo.astype(np.float32)
    if _return_res:
        return out, res
    return out


# revision 50
# speedup vs baseline: 1.0017x; 1.0017x over previous
"""Causal multi-head attention (B=4, S=2048, D=1024, H=16, hd=64) on 8 TRN2
NeuronCores.

Sharding: core c = (batch b = c//2, head-group g = c%2). Each core computes
QKV projections for its 8 heads (Megatron column-split), causal attention,
and a partial out-projection (row-split); the host sums the two head-group
partials per batch and adds the bias.

On-device layout (bf16 compute, fp32 PSUM accumulation):
  xT  [p, q-block, din-subtile, 512]  x[b]^T pre-tiled on host so each
        input DMA moves 8KB-contiguous runs per partition (descriptor-
        efficient); same for wq/wk (head-pair-blocked), wv, wo
  q/k projections in fp8-e4m3 DoubleRow (weights pre-scaled x64 on host,
        the exp scale absorbs the 1/4096; value path stays bf16 - fp8
        anywhere in v/attn costs ~1% extra rel err, over budget)
  qT/kT as [d_g, S] transposed tiles: head-pair t -> partitions
        [0:64] head 2t, [64:128] head 2t+1
  v   [k-tile 128, 8 heads, 65]: col 64 is ones (sumexp lands in the ctx^T
        psum row 64 for free during the attn*V matmul)
  scores^T psum tiles [k 128, 2 heads, q 512]: head pair packed via PE row
        tiling (K=64 each, concurrent).  Scores for TWO k-tiles are emitted
        back-to-back: full-array<->row-group LDWEIGHTS transitions stall
        ~100ns each (the PE can only pull an LDW ahead of in-flight matmuls
        into a non-conflicting row group), so batching the row-tiled pairs
        halves the number of transitions.
  attn = exp(scores/8) per k-tile on ScalarE; causal via skipping k-tiles
        above the diagonal, restricting the q-range on diagonal tiles, and
        one masked multiply per diagonal tile (the 128-wide diagonal block
        is the same upper triangle for every dd, both heads in one op)
  ctx^T accumulated in PSUM over k-tiles; normalize via DRAM-roundtrip
        reciprocal + gpsimd partition broadcast.  The last chunk instead
        transposes the sumexp row straight out of PSUM (DVE 32x32 block
        transpose), reciprocates in place, transposes back, and multiplies
        straight from PSUM - no SBUF copies on the critical tail.

Schedule: all non-attention matmuls drain as filler INSIDE the attention
stream (interpolated between per-chunk prerequisite markers); attn*V is
emitted one k-tile-PAIR late so its exp is always ready; ~10 short dummy
matmuls keep HAM at full clock through the initial DMA wait without
head-blocking the prefill; input DMAs are spread over the sync, scalar and
gpsimd rings so the scalar queue is clear before the exp stream starts;
row-3 out-projection is split so only one matmul + add + DMA per o-tile
remains after the final normalize, pipelined over psum/sbuf double-buffers
and both DMA rings.
"""

import numpy as np
import ml_dtypes

import concourse.bass as bass
import concourse.tile as tile
from concourse import bacc, mybir
from concourse.bass_utils import run_bass_kernel_spmd

P = 128          # partitions
S = 2048         # sequence length (one batch per core)
DIN = 1024       # model dim
DG = 512         # head-group width per core (8 heads x 64)
HD = 64          # head dim
NH = 8           # heads per core
QC = 512         # q-chunk (matmul free dim)
NQC = S // QC    # 4 q-chunks
NKT = S // P     # 16 k-tiles
KDT = DIN // P   # 8 din k-tiles
NHP = 4          # head pairs per core
F32 = mybir.dt.float32
BF16 = mybir.dt.bfloat16
FP8 = mybir.dt.float8e4
EXP = mybir.ActivationFunctionType.Exp
DR = mybir.MatmulPerfMode.DoubleRow

WSCALE = 64.0    # fp8 q/k weight pre-scale
N_WARM = 10      # dummy warm-up matmuls during the initial DMA wait; must
WARM_N = 512     # sustain >3.4us of PE activity to trip HAM to full clock
N_TAILWARM = 20  # dummy matmuls covering the last-normalize PE gap

_CACHE = {}


def _emit(tc, d):
    nc = tc.nc
    with (
        nc.allow_low_precision(reason="bf16 attention pipeline"),
        tc.tile_pool(name="persist", bufs=1) as pp,
        tc.tile_pool(name="work", bufs=4) as wp,
        tc.tile_pool(name="psc", bufs=2, space="PSUM") as psc,
        tc.tile_pool(name="ppj", bufs=2, space="PSUM") as ppj,
        tc.tile_pool(name="pcx", bufs=1, space="PSUM") as pcx,
    ):
        # ---- persistent SBUF tiles (layouts match the pre-tiled DRAM) ----
        xT = pp.tile([P, NQC, KDT, QC], BF16, tag="xT", name="xT")
        x8 = pp.tile([P, NQC, KDT, QC], FP8, tag="x8", name="x8")
        wq = pp.tile([P, NHP, KDT, P], FP8, tag="wq", name="wq")
        wk = pp.tile([P, NHP, KDT, P], FP8, tag="wk", name="wk")
        wv = pp.tile([P, KDT, DG], BF16, tag="wv", name="wv")
        wo = pp.tile([P, 4, DIN], BF16, tag="wo", name="wo")
        qT = [pp.tile([P, S], BF16, tag=f"qT{t}", name=f"qT{t}") for t in range(NHP)]
        kT = [pp.tile([P, S], BF16, tag=f"kT{t}", name=f"kT{t}") for t in range(NHP)]
        vv = [pp.tile([P, NH, HD + 1], BF16, tag=f"v{m}", name=f"v{m}") for m in range(NKT)]
        cx = [pp.tile([P, S], BF16, tag=f"cx{t}", name=f"cx{t}") for t in range(NHP)]
        ob3 = pp.tile([P, 8, QC], BF16, tag="ob3", name="ob3")
        msk = pp.tile([P, 2, P], BF16, tag="msk", name="msk")
        idt = pp.tile([P, P], BF16, tag="idt", name="idt")
        wrm = pp.tile([P, WARM_N], BF16, tag="wrm", name="wrm")

        # ---- PE warm-up: garbage matmuls (the psum is never read) keep the
        # PE busy from the earliest possible moment so HAM reaches full
        # clock before the first chains, without committing the PE FIFO
        # much past the x8 arrival ----
        nc.vector.memset(wrm[:], 0.0)
        for g in range(0, N_WARM, 5):
            ps = ppj.tile([P, WARM_N], F32, tag="pj", name="ps")
            n = min(5, N_WARM - g)
            for i in range(n):
                nc.tensor.matmul(
                    ps[:], wrm[:, 0:P], wrm[:],
                    start=(i == 0), stop=(i == n - 1),
                )

        # ---- input DMAs: big contiguous-run transfers, ordered by need,
        # critical prefill set first, spread across three HW-DGE rings so
        # the scalar queue is idle before the exp stream starts ----
        # quarter-granular x8 chunk: each quarter is exactly one DR matmul's
        # operand, so the prefill chains start as soon as the FIRST quarter
        # lands instead of waiting for a 256KB half
        for qq in range(0, KDT, 2):
            nc.sync.dma_start(x8[:, 0, qq:qq + 2, :], d["xq"][:, 0, qq:qq + 2, :])
        nc.scalar.dma_start(wq[:, 0, :, :], d["wqT"][:, 0, :, :])
        nc.scalar.dma_start(wk[:, 0, :, :], d["wkT"][:, 0, :, :])
        nc.scalar.dma_start(msk[:], d["masks"][:])
        nc.sync.dma_start(wv[:], d["wvT"][:])
        nc.sync.dma_start(xT[:, 0, :, :], d["xT"][:, 0, :, :])
        nc.scalar.dma_start(wq[:, 1:NHP, :, :], d["wqT"][:, 1:NHP, :, :])
        nc.scalar.dma_start(wk[:, 1:NHP, :, :], d["wkT"][:, 1:NHP, :, :])
        nc.scalar.dma_start(idt[:], d["ident"][:])
        # deferred inputs ride the scalar ring BEHIND the critical weights:
        # the serial ring is a priority queue, while a separate (empty) ring
        # would fire immediately and steal HBM bandwidth from the critical
        # x8/xT stream on the sync ring
        for s in range(1, NQC):
            nc.sync.dma_start(xT[:, s, :, :], d["xT"][:, s, :, :])
            nc.scalar.dma_start(x8[:, s, :, :], d["xq"][:, s, :, :])
        nc.scalar.dma_start(wo[:], d["woT"][:])

        # ---- filler units ----
        def u_v(m):
            def f():
                ps = ppj.tile([P, QC], F32, tag="pj", name="ps")
                for k in range(KDT):
                    nc.tensor.matmul(
                        ps[:],
                        xT[:, m // 4, k, (m % 4) * P:(m % 4 + 1) * P],
                        wv[:, k, :],
                        start=(k == 0),
                        stop=(k == KDT - 1),
                    )
                nc.vector.tensor_copy(
                    vv[m][:, :, 0:HD], ps[:].rearrange("p (h e) -> p h e", h=NH)
                )
                nc.vector.memset(vv[m][:, :, HD:HD + 1], 1.0)
            return f

        def u_chain(t, w, s):
            def f():
                wt, dst = ((wq, qT), (wk, kT))[w]
                ps = ppj.tile([P, QC], F32, tag="pj", name="ps")
                for k in range(0, KDT, 2):
                    nc.tensor.matmul(
                        ps[:],
                        wt[:, t, k:k + 2, :],
                        x8[:, s, k:k + 2, :],
                        start=(k == 0),
                        stop=(k == KDT - 2),
                        perf_mode=DR,
                    )
                nc.vector.tensor_copy(dst[t][:, s * QC:(s + 1) * QC], ps[:])
            return f

        def u_out(s, o):
            def f():
                ps = ppj.tile([P, QC], F32, tag="pj", name="ps")
                for k in range(4):
                    nc.tensor.matmul(
                        ps[:],
                        wo[:, k, o * P:(o + 1) * P],
                        cx[k][:, s * QC:(s + 1) * QC],
                        start=(k == 0), stop=(k == 3),
                    )
                ob = wp.tile([P, QC], BF16, tag="ob", name="ob", bufs=2)
                nc.vector.tensor_copy(ob[:], ps[:])
                nc.sync.dma_start(
                    d["outT"][o * P:(o + 1) * P, s * QC:(s + 1) * QC], ob[:]
                )
            return f

        def u_out3_partial(o):
            # row-3 out-proj, head-pair groups 0..2 only -> SBUF partial
            def f():
                ps = ppj.tile([P, QC], F32, tag="pj", name="ps")
                for k in range(3):
                    nc.tensor.matmul(
                        ps[:],
                        wo[:, k, o * P:(o + 1) * P],
                        cx[k][:, 3 * QC:S],
                        start=(k == 0), stop=(k == 2),
                    )
                nc.vector.tensor_copy(ob3[:, o, :], ps[:])
            return f

        def u_out3_final(t):
            # o-tile pair 2t/2t+1: two row-3 matmuls + ONE identity matmul
            # folding in the row-0..2 partials (PE is idle at the tail, DVE
            # is not); psum->SBUF copies alternate DVE/ScalarE so the two
            # copy chains run in parallel
            ps = psc.tile([P, 2, QC], F32, tag="sc", name="fin")
            for j in range(2):
                o = 2 * t + j
                nc.tensor.matmul(
                    ps[:, j, :], wo[:, 3, o * P:(o + 1) * P], cx[3][:, 3 * QC:S],
                    start=True, stop=False,
                )
                nc.tensor.matmul(
                    ps[:, j, :], idt[:], ob3[:, o, :], start=False, stop=True,
                )
            # half-granular copies so each o-tile's DMA fires as soon as its
            # half of the psum is copied out (copy chains alternate engines)
            ob = wp.tile([P, 2, QC], BF16, tag="obf", name="obf", bufs=2)
            eng = nc.sync if t % 2 == 0 else nc.gpsimd
            for j in range(2):
                o = 2 * t + j
                if t % 2 == 0:
                    nc.vector.tensor_copy(ob[:, j, :], ps[:, j, :])
                else:
                    nc.scalar.activation(
                        ob[:, j, :], ps[:, j, :],
                        mybir.ActivationFunctionType.Copy,
                    )
                eng.dma_start(
                    d["outT"][o * P:(o + 1) * P, 3 * QC:S], ob[:, j, :]
                )

        # consume-ordered filler queue + hard prerequisites per chunk
        queue = [u_v(0), u_v(1), u_v(2), u_v(3)]
        pre = {}
        for s in range(NQC):
            for hp in range(NHP):
                if (hp, s) == (0, 0):
                    pre[(hp, s)] = 0
                    continue
                if hp == 0 and s >= 1:
                    queue += [u_v(m) for m in range(4 * s, 4 * s + 4)]
                queue += [u_chain(hp, 0, s), u_chain(hp, 1, s)]
                pre[(hp, s)] = len(queue)
            if s == 1 or s == 2:
                queue += [u_out(s - 1, o) for o in range(8)]
        queue += [u_out(2, o) for o in range(8)]
        queue += [u_out3_partial(o) for o in range(8)]
        n_units = len(queue)

        order = [(hp, s) for s in range(NQC) for hp in range(NHP)]
        nxt = {order[i]: order[i + 1] for i in range(len(order) - 1)}

        state = {"drained": 0}

        def drain_to(idx):
            while state["drained"] < idx:
                queue[state["drained"]]()
                state["drained"] += 1

        def normalize(hp, s):
            last = (hp, s) == (NHP - 1, NQC - 1)
            cps = state["cps"]
            if last:
                # low-latency tail: DVE 32x32 block-transpose lifts the
                # [1,1024] sumexp row (row 64 = col 0 of the 32-aligned psum
                # window [64:96]; rows 65:96 are memset filler) onto 32
                # partitions straight out of PSUM, reciprocal in place
                # (col 0 -> col 1), transpose back, then multiply straight
                # from PSUM - no SBUF staging on the critical tail
                t1 = wp.tile([32, 2, QC], F32, tag="t1", name="t1", bufs=1)
                t2 = wp.tile([32, 2, QC], F32, tag="t2", name="t2", bufs=1)
                nc.vector.memset(t2[:], 1.0)  # early, off the critical path
                nc.vector.transpose(t1[:], cps[64:96, :, :])
                tv1 = t1[:].rearrange("p h (b j) -> p h b j", j=32)
                tv2 = t2[:].rearrange("p h (b j) -> p h b j", j=32)
                nc.vector.reciprocal(tv2[:, :, :, 0:1], tv1[:, :, :, 0:1])
                rc = wp.tile([32, 2, QC], F32, tag="rc", name="rc", bufs=1)
                nc.vector.transpose(rc[:], t2[:])
                bs = wp.tile([HD, 2, QC], F32, tag="bs", name="bs", bufs=2)
                nc.gpsimd.partition_broadcast(bs[:], rc[0:1, :, :])
                state["bs"] = bs
                cxs = wp.tile([HD, QC], BF16, tag="cxs", name="cxs", bufs=2)
                nc.vector.tensor_mul(cxs[:], cps[0:HD, 1, :], bs[:, 1, :])
                nc.sync.dma_start(cx[hp][HD:P, s * QC:(s + 1) * QC], cxs[:])
                nc.vector.tensor_mul(
                    cx[hp][0:HD, s * QC:(s + 1) * QC], cps[0:HD, 0, :], bs[:, 0, :]
                )
                return
            cb = wp.tile([HD + 1, 2, QC], F32, tag="cb", name="cb", bufs=2)
            nc.vector.tensor_copy(cb[:], cps[0:HD + 1, :, :])
            zt = wp.tile([P, 8], F32, tag="zt", name="zt", bufs=2)
            nc.sync.dma_start(zt[:], cb[HD:HD + 1, :, :])
            rt = wp.tile([P, 8], F32, tag="rt", name="rt", bufs=2)
            nc.vector.reciprocal(rt[:], zt[:])
            rr = wp.tile([1, 2, QC], F32, tag="rr", name="rr", bufs=2)
            nc.sync.dma_start(rr[:], rt[:])
            bs = wp.tile([HD, 2, QC], F32, tag="bs", name="bs", bufs=2)
            nc.gpsimd.partition_broadcast(bs[:], rr[:])
            # head B first: its partition-shift DMA overlaps head A's mul
            cxs = wp.tile([HD, QC], BF16, tag="cxs", name="cxs", bufs=2)
            nc.vector.tensor_mul(cxs[:], cb[0:HD, 1, :], bs[:, 1, :])
            nc.sync.dma_start(cx[hp][HD:P, s * QC:(s + 1) * QC], cxs[:])
            nc.vector.tensor_mul(
                cx[hp][0:HD, s * QC:(s + 1) * QC], cb[0:HD, 0, :], bs[:, 0, :]
            )

        def attn_chunk(hp, s):
            t0 = pre[(hp, s)]
            t1 = pre[nxt[(hp, s)]] if (hp, s) in nxt else n_units
            nkt = 4 * (s + 1)  # causal: k-tiles 0..nkt-1
            cps = pcx.tile([96, 2, QC], F32, tag="cx", name="cps")
            state["cps"] = cps
            if (hp, s) == (NHP - 1, NQC - 1):
                # valid filler above the sumexp row for the tail transpose
                # (32-aligned window; row 64 is re-written by the start=True
                # attn*V accumulation right after)
                nc.vector.memset(cps[HD:96, :, :], 1.0)

            def attn_v_pair(pair):
                for k, s0, a in pair:
                    nc.tensor.matmul(
                        cps[0:HD + 1, 0, s0:], vv[k][:, 2 * hp, :], a[:, 0, s0:],
                        start=(k == 0), stop=(k == nkt - 1),
                    )
                    nc.tensor.matmul(
                        cps[0:HD + 1, 1, s0:], vv[k][:, 2 * hp + 1, :], a[:, 1, s0:],
                        start=(k == 0), stop=(k == nkt - 1),
                    )

            pend = None  # attn*V emitted one k-tile-PAIR late: its exp and
            # mask are always done by the time it reaches the head of the
            # FIFO tensor queue, so it never head-blocks the scores stream
            for pk in range(nkt // 2):
                cur = []
                for k in (2 * pk, 2 * pk + 1):
                    dd = k - 4 * s
                    s0 = max(dd, 0) * P  # causal q-range restriction
                    sps = psc.tile([P, 2, QC], F32, tag="sc", name="sps")
                    nc.tensor.matmul(
                        sps[:, 0, s0:],
                        kT[hp][0:HD, k * P:(k + 1) * P],
                        qT[hp][0:HD, s * QC + s0:(s + 1) * QC],
                        start=True, stop=True,
                    )
                    nc.tensor.matmul(
                        sps[:, 1, s0:],
                        kT[hp][HD:P, k * P:(k + 1) * P],
                        qT[hp][HD:P, s * QC + s0:(s + 1) * QC],
                        start=True, stop=True,
                    )
                    a = wp.tile([P, 2, QC], BF16, tag="a", name="a", bufs=6)
                    nc.scalar.activation(
                        a[:, :, s0:], sps[:, :, s0:], EXP, scale=d["escale"]
                    )
                    if dd >= 0:
                        # only columns [s0, s0+128) straddle the diagonal;
                        # the block mask is the same triangle for every dd
                        nc.vector.tensor_mul(
                            a[:, :, s0:s0 + P], a[:, :, s0:s0 + P], msk[:]
                        )
                    cur.append((k, s0, a))
                if pend is not None:
                    attn_v_pair(pend)
                pend = cur
                drain_to(min(t1, t0 + ((t1 - t0) * (2 * pk + 4)) // nkt,
                             state["drained"] + 5))
            attn_v_pair(pend)
            normalize(hp, s)

        # ---- prefill: just enough to start chunk (0,0) ----
        u_chain(0, 0, 0)()
        u_chain(0, 1, 0)()

        # ---- main stream ----
        for s in range(NQC):
            for hp in range(NHP):
                drain_to(pre[(hp, s)])
                attn_chunk(hp, s)
        drain_to(n_units)
        # dummy matmuls bridge the PE gap while the final normalize runs so
        # HAM stays at full clock for the final out-projection row
        # the first dummy groups use the scores psum ring (free right after
        # the last exp) so they are not serialized behind the last partials'
        # psum copies on the ppj ring
        for g in range(0, N_TAILWARM, 5):
            if g < 10:
                ps = psc.tile([P, 2, QC], F32, tag="sc", name="dum")
            else:
                ps = ppj.tile([P, QC], F32, tag="pj", name="dum")
            n = min(5, N_TAILWARM - g)
            for i in range(n):
                nc.tensor.matmul(
                    ps[:, 0, :] if g < 10 else ps[:], wrm[:, 0:P], wrm[:],
                    start=(i == 0), stop=(i == n - 1),
                )
        # two f32 garbage matmuls gated on the final broadcast: they become
        # ready exactly when it lands, bridging the remaining PE idle window
        # up to the final out-projection (keeps HAM at 8/8); emitted AFTER
        # the plain dummies so they never head-block the FIFO
        bs = state["bs"]
        for _ in range(3):
            pw = ppj.tile([P, QC], F32, tag="pj", name="dumb")
            nc.tensor.matmul(
                pw[:], bs[:, 0, 0:P], bs[:, 0, :], start=True, stop=True,
            )
        for t in range(4):
            u_out3_final(t)


def _build():
    if "nc" in _CACHE:
        return _CACHE["nc"]
    nc = bacc.Bacc("TRN2", target_bir_lowering=False, debug=False, num_devices=8)
    d = {
        "xT": nc.dram_tensor("xT", [P, NQC, KDT, QC], BF16, kind="ExternalInput").ap(),
        "wvT": nc.dram_tensor("wvT", [P, KDT, DG], BF16, kind="ExternalInput").ap(),
        "woT": nc.dram_tensor("woT", [P, 4, DIN], BF16, kind="ExternalInput").ap(),
        "masks": nc.dram_tensor("masks", [P, 2, P], BF16, kind="ExternalInput").ap(),
        "ident": nc.dram_tensor("ident", [P, P], BF16, kind="ExternalInput").ap(),
        "outT": nc.dram_tensor("outT", [DIN, S], BF16, kind="ExternalOutput").ap(),
        "wqT": nc.dram_tensor("wqT", [P, NHP, KDT, P], FP8, kind="ExternalInput").ap(),
        "wkT": nc.dram_tensor("wkT", [P, NHP, KDT, P], FP8, kind="ExternalInput").ap(),
        "xq": nc.dram_tensor("xq", [P, NQC, KDT, QC], FP8, kind="ExternalInput").ap(),
        "escale": 0.125 / (WSCALE * WSCALE),
    }
    with tile.TileContext(nc) as tc:
        _emit(tc, d)
    nc.compile()
    _CACHE["nc"] = nc
    return nc


def _masks_np():
    r = np.arange(P)[:, None]
    j = np.arange(P)[None, :]
    m = (j >= r).astype(ml_dtypes.bfloat16)  # [128, 128] upper triangle
    return np.ascontiguousarray(np.broadcast_to(m[:, None, :], (P, 2, P)))


def _tile_k(a, kdt=KDT):
    """[kdt*P, C] -> [P, kdt, C] (din-subtile blocking)."""
    c = a.shape[1]
    return np.ascontiguousarray(a.reshape(kdt, P, c).transpose(1, 0, 2))


def _f8(a):
    return np.clip(a, -240, 240).astype(ml_dtypes.float8_e4m3)


def kernel(x, Wq, Wk, Wv, Wo, bo, _run_kwargs=None, _return_res=False):
    x = np.asarray(x)
    Wq, Wk, Wv, Wo, bo = (np.asarray(a) for a in (Wq, Wk, Wv, Wo, bo))
    B = x.shape[0]
    nc = _build()

    def b16(a):
        return np.ascontiguousarray(a).astype(ml_dtypes.bfloat16)

    masks = _masks_np()
    in_maps = []
    for c in range(8):
        b, g = divmod(c, 2)
        xt = b16(x[b].T)  # [1024, 2048]
        xt4 = xt.reshape(KDT, P, NQC, QC).transpose(1, 2, 0, 3)  # [p,s,k,c]
        wqt = Wq[g * DG:(g + 1) * DG, :].T  # [1024, 512] f32
        wkt = Wk[g * DG:(g + 1) * DG, :].T
        im = {
            "xT": np.ascontiguousarray(xt4),
            "wvT": _tile_k(b16(Wv[g * DG:(g + 1) * DG, :].T)),
            "woT": _tile_k(b16(Wo[:, g * DG:(g + 1) * DG].T), kdt=4),
            "masks": masks,
            "ident": np.eye(P, dtype=ml_dtypes.bfloat16),
            "xq": np.ascontiguousarray(
                _f8(x[b].T).reshape(KDT, P, NQC, QC).transpose(1, 2, 0, 3)),
            "wqT": np.ascontiguousarray(
                _f8(WSCALE * wqt).reshape(KDT, P, NHP, P).transpose(1, 2, 0, 3)),
            "wkT": np.ascontiguousarray(
                _f8(WSCALE * wkt).reshape(KDT, P, NHP, P).transpose(1, 2, 0, 3)),
        }
        in_maps.append(im)

    res = run_bass_kernel_spmd(nc, in_maps, list(range(8)), **(_run_kwargs or {}))
    out = np.empty((B, S, DIN), np.float32)
    for b in range(B):
        p = (res.results[2 * b]["outT"].astype(np.float32)
             + res.results[2 * b + 1]["outT"].astype(np.float32))
        out[b] = p.T + bo.astype(np.float32)
    if _return_res:
        return out, res
    return out
